# revision 7
# baseline (speedup 1.0000x reference)
"""Trainium2 Bass kernel for nn_DNDecoder (GNN edge-MLP decoder).

out[e] = W2 @ LeakyReLU(W1 @ [z[row_e]; z[col_e]] + b1) + b2   for 1.6M edges.

Strategy (8 NeuronCores, edges sharded data-parallel):
  - z is cast to fp16 and replicated on every core; per-edge node features are
    fetched with GPSIMD dma_gather in NON-transpose mode (row-major landing:
    edge j of a segment -> partition j%128, block j//128). Non-transpose
    gathers do not touch the shared transpose crossbar, so they run CLEANLY on
    all 4 SWDGE queues concurrently (transpose-mode gathers corrupt each other
    across queues). 4 queues = 4 Q7 core pairs generating descriptors in
    parallel: ~2.0 ns/idx vs 6.3 ns/idx single-queue.
  - dma_gather indices are int16 (<32768), so nodes are split into 4 windows
    of 32768; each core's edges are sorted into the 16 (row-window,
    col-window) groups. Group capacities are shared across cores so one SPMD
    program serves all 8 cores. Output is un-permuted on the host.
  - Each [128 edge, 128 feat] gathered block is PE-transposed (fp16
    pass-through matmul vs identity) into PSUM, copied to SBUF by the vector
    engine, then the usual pipeline: PSUM U = W1aT.T@zr + W1bT.T@zc (2 fp16
    matmuls), one ScalarE activation does bias + LeakyReLU + fp16 cast, and a
    third matmul with W2 embedded at the tile's column of a [128,32]
    stationary accumulates 128 tiles of final dot products into one PSUM bank
    ([128 tiles, 512 lanes]), copied out once per 65536 edges.
  - single_packet=True wedges the device; keep False. Multi-queue transpose
    gathers corrupt via the shared xbar; non-transpose verified clean.
"""

import sys

for p in ("/opt/trn_rl_repo", "/opt/pypackages"):
    if p not in sys.path:
        sys.path.append(p)

import numpy as np

N_NODES = 100000
H = 128
E_TOTAL = 1600000
N_CORES = 8
EDGES_PER_CORE = E_TOTAL // N_CORES
BWIN = 32768          # index window (int16 gather limit)
NB = 4                # node windows
TILE = 512            # edges per matmul tile (one PSUM bank)
SUP = 128             # tiles per output supertile (one PSUM bank of results)
GATHER_N = 3072       # max edges per dma_gather call (193 descs/engine < ring)
NQ = 4                # SWDGE queues (non-transpose gathers are multi-queue clean)


def _plan(row, col):
    """Shared-structure plan across cores.

    Returns (caps, segments, NTILES, NSUP) and per-core
    (order, local_r, local_c) where order[i] = original edge position of the
    i-th edge in group-sorted order.
    """
    per_core = []
    sizes = np.zeros((N_CORES, NB * NB), np.int64)
    for c in range(N_CORES):
        r = row[c * EDGES_PER_CORE:(c + 1) * EDGES_PER_CORE]
        cc = col[c * EDGES_PER_CORE:(c + 1) * EDGES_PER_CORE]
        g = (r >> 15) * NB + (cc >> 15)
        order = np.argsort(g, kind="stable")
        gs = g[order]
        sizes[c] = np.bincount(g, minlength=NB * NB)
        per_core.append((order, r[order], cc[order], gs))
    caps = ((sizes.max(axis=0) + TILE - 1) // TILE) * TILE
    segments = []  # (group, n, tot_off)
    off = 0
    for g in range(NB * NB):
        rem = int(caps[g])
        while rem > 0:
            n = min(GATHER_N, rem)
            segments.append((g, n, off))
            off += n
            rem -= n
    tot = off
    ntiles = tot // TILE
    nsup = (ntiles + SUP - 1) // SUP
    return caps, segments, tot, ntiles, nsup, per_core


def _wrap_idx(local_idx, tot):
    """Pack segment-relative int16 indices into the [128, tot//16] wrapped
    layout dma_gather expects (16-partition wrap, replicated 8x)."""
    a16 = local_idx.reshape(-1, 16).T.astype(np.int16)  # [16, tot//16]
    return np.tile(a16, (8, 1))


def build_program(ntiles, nsup, segments, b2val, reps=1, mode="full", nqueues=NQ):
    import concourse.bass as bass
    import concourse.mybir as mybir
    from concourse import bacc
    from concourse.tile import TileContext

    nc = bacc.Bacc(None, target_bir_lowering=False, debug=False,
                   num_swdge_queues=nqueues)
    tot = segments[-1][1] + segments[-1][2]

    zwins = []
    for w in range(NB):
        wn = min(BWIN, N_NODES - w * BWIN)
        zwins.append(nc.declare_dram_parameter(f"zw{w}", [wn, H], mybir.dt.float16, isOutput=False))
    idx_r = nc.declare_dram_parameter("idx_r", [128, tot // 16], mybir.dt.int16, isOutput=False)
    idx_c = nc.declare_dram_parameter("idx_c", [128, tot // 16], mybir.dt.int16, isOutput=False)
    w1aT = nc.declare_dram_parameter("w1aT", [128, 128], mybir.dt.float16, isOutput=False)
    w1bT = nc.declare_dram_parameter("w1bT", [128, 128], mybir.dt.float16, isOutput=False)
    b1d = nc.declare_dram_parameter("b1d", [128, 1], mybir.dt.float32, isOutput=False)
    w2e = nc.declare_dram_parameter("w2e", [128, 32 * 32], mybir.dt.float16, isOutput=False)
    identd = nc.declare_dram_parameter("identd", [128, 128], mybir.dt.float16, isOutput=False)
    outd = nc.declare_dram_parameter("out", [nsup, 128, TILE], mybir.dt.float32, isOutput=True)

    GB = GATHER_N // 128  # gather tile blocks

    with TileContext(nc) as tc:
        with (
            tc.tile_pool(name="const", bufs=1) as cpool,
            tc.tile_pool(name="gath", bufs=4) as gpool,
            tc.tile_pool(name="sp", bufs=4) as spool,
            tc.tile_pool(name="yp", bufs=3) as ypool,
            tc.tile_pool(name="op", bufs=2) as opool,
            tc.tile_pool(name="trp", bufs=2, space="PSUM") as trp,
            tc.tile_pool(name="upsum", bufs=2, space="PSUM") as upp,
            tc.tile_pool(name="opsum", bufs=2, space="PSUM") as opp,
        ):
            w1a_t = cpool.tile([128, 128], mybir.dt.float16, tag="w1a")
            nc.sync.dma_start(out=w1a_t[:], in_=w1aT[:])
            w1b_t = cpool.tile([128, 128], mybir.dt.float16, tag="w1b")
            nc.sync.dma_start(out=w1b_t[:], in_=w1bT[:])
            b1_t = cpool.tile([128, 1], mybir.dt.float32, tag="b1")
            nc.sync.dma_start(out=b1_t[:], in_=b1d[:])
            w2e_t = cpool.tile([128, 32 * 32], mybir.dt.float16, tag="w2e")
            nc.sync.dma_start(out=w2e_t[:], in_=w2e[:])
            ident_t = cpool.tile([128, 128], mybir.dt.float16, tag="ident")
            nc.sync.dma_start(out=ident_t[:], in_=identd[:])
            # whole index tensors resident in SBUF (26KB/partition each)
            ir_all = cpool.tile([128, tot // 16], mybir.dt.int16, tag="ir")
            nc.sync.dma_start(out=ir_all[:], in_=idx_r[:])
            ic_all = cpool.tile([128, tot // 16], mybir.dt.int16, tag="ic")
            nc.sync.dma_start(out=ic_all[:], in_=idx_c[:])

            def body(_=None):
                T = 0
                gq = 0
                out_ps = None
                for si, (g, n, off) in enumerate(segments):
                    zw_r = zwins[g // NB]
                    zw_c = zwins[g % NB]
                    o16 = off // 16
                    n16 = n // 16
                    nb = n // 128
                    zr = gpool.tile([128, GB, H], mybir.dt.float16, tag="zr")
                    zc = gpool.tile([128, GB, H], mybir.dt.float16, tag="zc")
                    nc.gpsimd.dma_gather(zr[:, :nb, :], zw_r[:, :],
                                         ir_all[:, o16:o16 + n16],
                                         n, n, H, transpose=False, single_packet=False,
                                         queue_num=gq % nqueues)
                    gq += 1
                    nc.gpsimd.dma_gather(zc[:, :nb, :], zw_c[:, :],
                                         ic_all[:, o16:o16 + n16],
                                         n, n, H, transpose=False, single_packet=False,
                                         queue_num=gq % nqueues)
                    gq += 1
                    if mode == "gather":
                        continue
                    for t in range(n // TILE):
                        S, pos = T // SUP, T % SUP
                        strip, k = pos // 32, pos % 32
                        if pos == 0:
                            out_ps = opp.tile([128, TILE], mybir.dt.float32, tag="ops")
                        # PE-transpose 4 blocks of each side into fp16 PSUM
                        trr = trp.tile([128, TILE], mybir.dt.float16, tag="trr")
                        trc = trp.tile([128, TILE], mybir.dt.float16, tag="trc")
                        for b in range(4):
                            nc.tensor.transpose(trr[:, b * 128:(b + 1) * 128],
                                                zr[:, 4 * t + b, :], ident_t[:])
                        for b in range(4):
                            nc.tensor.transpose(trc[:, b * 128:(b + 1) * 128],
                                                zc[:, 4 * t + b, :], ident_t[:])
                        zr_s = spool.tile([128, TILE], mybir.dt.float16, tag="zrs")
                        nc.vector.tensor_copy(zr_s[:], trr[:])
                        zc_s = spool.tile([128, TILE], mybir.dt.float16, tag="zcs")
                        nc.scalar.copy(zc_s[:], trc[:])
                        u_ps = upp.tile([128, TILE], mybir.dt.float32, tag="u")
                        nc.tensor.matmul(u_ps[:], w1a_t[:], zr_s[:], start=True, stop=False)
                        nc.tensor.matmul(u_ps[:], w1b_t[:], zc_s[:], start=False, stop=True)
                        y = ypool.tile([128, TILE], mybir.dt.float16, tag="y")
                        nc.scalar.activation(y[:], u_ps[:], mybir.ActivationFunctionType.Lrelu,
                                             bias=b1_t[:], scale=1.0, alpha=0.01)
                        last_in_strip = (k == 31) or (T == ntiles - 1)
                        nc.tensor.matmul(
                            out_ps[32 * strip:32 * (strip + 1), :],
                            w2e_t[:, 32 * k:32 * (k + 1)],
                            y[:],
                            start=(k == 0), stop=last_in_strip,
                            tile_position=(0, 32 * strip),
                        )
                        T += 1
                        if pos == SUP - 1 or T == ntiles:
                            rows = 32 * (strip + 1)
                            o_sb = opool.tile([128, TILE], mybir.dt.float32, tag="osb")
                            nc.vector.tensor_scalar_add(o_sb[:rows, :], out_ps[:rows, :], float(b2val))
                            nc.sync.dma_start(out=outd[S, 0:rows, :], in_=o_sb[:rows, :])

            if reps == 1:
                body()
            else:
                with tc.For_i(0, reps, 1) as _i:
                    body(_i)

    nc.compile()
    return nc


def prepare_inputs(z, edge_label_index, W1, b1, W2):
    z16 = np.asarray(z).astype(np.float16)
    eli = np.asarray(edge_label_index)
    row = eli[0].astype(np.int64)
    col = eli[1].astype(np.int64)
    caps, segments, tot, ntiles, nsup, per_core = _plan(row, col)

    W1 = np.asarray(W1, np.float32)
    w1aT16 = np.ascontiguousarray(W1[:, :H].T).astype(np.float16)
    w1bT16 = np.ascontiguousarray(W1[:, H:].T).astype(np.float16)
    b1_col = np.asarray(b1, np.float32).reshape(128, 1)
    w2_16 = np.asarray(W2, np.float32)[0].astype(np.float16)
    w2e_np = np.zeros((128, 32 * 32), np.float16)
    for k in range(32):
        w2e_np[:, 32 * k + k] = w2_16
    ident_np = np.eye(128, dtype=np.float16)

    group_start = np.zeros(NB * NB, np.int64)
    group_start[1:] = np.cumsum(caps)[:-1]

    in_maps = []
    scatter = []  # (order, valid_positions) per core
    for c in range(N_CORES):
        order, r_s, c_s, gs = per_core[c]
        # padded local indices, default 0 (gathers window base, discarded)
        lr = np.zeros(tot, np.int16)
        lc = np.zeros(tot, np.int16)
        sizes = np.bincount(gs, minlength=NB * NB)
        valid_pos = np.empty(EDGES_PER_CORE, np.int64)
        cur = 0
        for g in range(NB * NB):
            sgz = int(sizes[g])
            if sgz == 0:
                continue
            pos = group_start[g] + np.arange(sgz)
            lr[pos] = (r_s[cur:cur + sgz] - (g // NB) * BWIN).astype(np.int16)
            lc[pos] = (c_s[cur:cur + sgz] - (g % NB) * BWIN).astype(np.int16)
            valid_pos[cur:cur + sgz] = pos
            cur += sgz
        im_wins = {f"zw{w}": z16[w * BWIN:min((w + 1) * BWIN, N_NODES)] for w in range(NB)}
        in_maps.append({
            **im_wins,
            "idx_r": _wrap_idx(lr, tot),
            "idx_c": _wrap_idx(lc, tot),
            "w1aT": w1aT16, "w1bT": w1bT16,
            "b1d": b1_col, "w2e": w2e_np, "identd": ident_np,
        })
        scatter.append((order, valid_pos))
    return in_maps, scatter, segments, tot, ntiles, nsup


def assemble_output(results, scatter, nsup):
    out = np.empty(E_TOTAL, np.float32)
    for c in range(N_CORES):
        dev = results[c]["out"].reshape(nsup * 128 * TILE)
        order, valid_pos = scatter[c]
        oc = np.empty(EDGES_PER_CORE, np.float32)
        oc[order] = dev[valid_pos]
        out[c * EDGES_PER_CORE:(c + 1) * EDGES_PER_CORE] = oc
    return out


def kernel(z, edge_label_index, W1, b1, W2, b2):
    from concourse.bass_utils import run_bass_kernel_spmd

    in_maps, scatter, segments, tot, ntiles, nsup = prepare_inputs(
        z, edge_label_index, W1, b1, W2)
    b2val = float(np.asarray(b2).reshape(-1)[0])
    nc = build_program(ntiles, nsup, segments, b2val, reps=1)
    res = run_bass_kernel_spmd(nc, in_maps, list(range(N_CORES)))
    return assemble_output(res.results, scatter, nsup)


# revision 9
# speedup vs baseline: 1.2259x; 1.2259x over previous
"""Trainium2 Bass kernel for nn_DNDecoder (GNN edge-MLP decoder).

out[e] = W2 @ LeakyReLU(W1 @ [z[row_e]; z[col_e]] + b1) + b2   for 1.6M edges.

Strategy (8 NeuronCores, edges sharded data-parallel):
  - z is cast to fp16 and replicated on every core; per-edge node features are
    fetched with GPSIMD dma_gather in NON-transpose mode (row-major landing:
    edge j of a segment -> partition j%128, block j//128). Non-transpose
    gathers do not touch the shared transpose crossbar, so they run CLEANLY on
    all 4 SWDGE queues concurrently (transpose-mode gathers corrupt each other
    across queues). 4 queues = 4 Q7 core pairs generating descriptors in
    parallel: ~2.0 ns/idx vs 6.3 ns/idx single-queue.
  - dma_gather indices are int16 (<32768), so nodes are split into 4 windows
    of 32768; each core's edges are sorted into the 16 (row-window,
    col-window) groups. Group capacities are shared across cores so one SPMD
    program serves all 8 cores. Output is un-permuted on the host.
  - Each [128 edge, 128 feat] gathered block is PE-transposed (fp16
    pass-through matmul vs identity) into PSUM, copied to SBUF by the vector
    engine, then the usual pipeline: PSUM U = W1aT.T@zr + W1bT.T@zc (2 fp16
    matmuls), one ScalarE activation does bias + LeakyReLU + fp16 cast, and a
    third matmul with W2 embedded at the tile's column of a [128,32]
    stationary accumulates 128 tiles of final dot products into one PSUM bank
    ([128 tiles, 512 lanes]), copied out once per 65536 edges.
  - single_packet=True wedges the device; keep False. Multi-queue transpose
    gathers corrupt via the shared xbar; non-transpose verified clean.
"""

import sys

for p in ("/opt/trn_rl_repo", "/opt/pypackages"):
    if p not in sys.path:
        sys.path.append(p)

import numpy as np

N_NODES = 100000
H = 128
E_TOTAL = 1600000
N_CORES = 8
EDGES_PER_CORE = E_TOTAL // N_CORES
BWIN = 32768          # index window (int16 gather limit)
NB = 4                # node windows
TILE = 512            # edges per matmul tile (one PSUM bank)
SUP = 128             # tiles per output supertile (one PSUM bank of results)
GATHER_N = 3584       # max edges per dma_gather call (225 descs/engine < ring)
NQ = 4                # SWDGE queues (non-transpose gathers are multi-queue clean)


def _plan(row, col):
    """Shared-structure plan across cores.

    Returns (caps, segments, NTILES, NSUP) and per-core
    (order, local_r, local_c) where order[i] = original edge position of the
    i-th edge in group-sorted order.
    """
    per_core = []
    sizes = np.zeros((N_CORES, NB * NB), np.int64)
    for c in range(N_CORES):
        r = row[c * EDGES_PER_CORE:(c + 1) * EDGES_PER_CORE]
        cc = col[c * EDGES_PER_CORE:(c + 1) * EDGES_PER_CORE]
        g = (r >> 15) * NB + (cc >> 15)
        order = np.argsort(g, kind="stable")
        gs = g[order]
        sizes[c] = np.bincount(g, minlength=NB * NB)
        per_core.append((order, r[order], cc[order], gs))
    caps = ((sizes.max(axis=0) + TILE - 1) // TILE) * TILE
    segments = []  # (group, n, tot_off)
    off = 0
    for g in range(NB * NB):
        rem = int(caps[g])
        while rem > 0:
            n = min(GATHER_N, rem)
            segments.append((g, n, off))
            off += n
            rem -= n
    tot = off
    ntiles = tot // TILE
    nsup = (ntiles + SUP - 1) // SUP
    return caps, segments, tot, ntiles, nsup, per_core


def _wrap_idx(local_idx, tot):
    """Pack segment-relative int16 indices into the [128, tot//16] wrapped
    layout dma_gather expects (16-partition wrap, replicated 8x)."""
    a16 = local_idx.reshape(-1, 16).T.astype(np.int16)  # [16, tot//16]
    return np.tile(a16, (8, 1))


def build_program(ntiles, nsup, segments, b2val, reps=1, mode="full", nqueues=NQ):
    import concourse.bass as bass
    import concourse.mybir as mybir
    from concourse import bacc
    from concourse.tile import TileContext

    nc = bacc.Bacc(None, target_bir_lowering=False, debug=False,
                   num_swdge_queues=nqueues)
    tot = segments[-1][1] + segments[-1][2]

    zwins = []
    for w in range(NB):
        wn = min(BWIN, N_NODES - w * BWIN)
        zwins.append(nc.declare_dram_parameter(f"zw{w}", [wn, H], mybir.dt.float16, isOutput=False))
    idx_r = nc.declare_dram_parameter("idx_r", [128, tot // 16], mybir.dt.int16, isOutput=False)
    idx_c = nc.declare_dram_parameter("idx_c", [128, tot // 16], mybir.dt.int16, isOutput=False)
    w1aT = nc.declare_dram_parameter("w1aT", [128, 128], mybir.dt.float16, isOutput=False)
    w1bT = nc.declare_dram_parameter("w1bT", [128, 128], mybir.dt.float16, isOutput=False)
    b1d = nc.declare_dram_parameter("b1d", [128, 1], mybir.dt.float32, isOutput=False)
    w2e = nc.declare_dram_parameter("w2e", [128, 32 * 32], mybir.dt.float16, isOutput=False)
    identd = nc.declare_dram_parameter("identd", [128, 128], mybir.dt.float16, isOutput=False)
    outd = nc.declare_dram_parameter("out", [nsup, 128, TILE], mybir.dt.float32, isOutput=True)

    GB = GATHER_N // 128  # gather tile blocks

    with TileContext(nc) as tc:
        with (
            tc.tile_pool(name="const", bufs=1) as cpool,
            tc.tile_pool(name="gath", bufs=4) as gpool,
            tc.tile_pool(name="sp", bufs=4) as spool,
            tc.tile_pool(name="yp", bufs=3) as ypool,
            tc.tile_pool(name="op", bufs=2) as opool,
            tc.tile_pool(name="trp", bufs=2, space="PSUM") as trp,
            tc.tile_pool(name="upsum", bufs=2, space="PSUM") as upp,
            tc.tile_pool(name="opsum", bufs=2, space="PSUM") as opp,
        ):
            w1a_t = cpool.tile([128, 128], mybir.dt.float16, tag="w1a")
            nc.sync.dma_start(out=w1a_t[:], in_=w1aT[:])
            w1b_t = cpool.tile([128, 128], mybir.dt.float16, tag="w1b")
            nc.sync.dma_start(out=w1b_t[:], in_=w1bT[:])
            b1_t = cpool.tile([128, 1], mybir.dt.float32, tag="b1")
            nc.sync.dma_start(out=b1_t[:], in_=b1d[:])
            w2e_t = cpool.tile([128, 32 * 32], mybir.dt.float16, tag="w2e")
            nc.sync.dma_start(out=w2e_t[:], in_=w2e[:])
            ident_t = cpool.tile([128, 128], mybir.dt.float16, tag="ident")
            nc.sync.dma_start(out=ident_t[:], in_=identd[:])
            # whole index tensors resident in SBUF (26KB/partition each)
            ir_all = cpool.tile([128, tot // 16], mybir.dt.int16, tag="ir")
            nc.sync.dma_start(out=ir_all[:], in_=idx_r[:])
            ic_all = cpool.tile([128, tot // 16], mybir.dt.int16, tag="ic")
            nc.sync.dma_start(out=ic_all[:], in_=idx_c[:])

            def body(_=None):
                T = 0
                qload = [0] * nqueues
                out_ps = None

                def next_q(n):
                    q = qload.index(min(qload))
                    qload[q] += n
                    return q

                for si, (g, n, off) in enumerate(segments):
                    zw_r = zwins[g // NB]
                    zw_c = zwins[g % NB]
                    o16 = off // 16
                    n16 = n // 16
                    nb = n // 128
                    zr = gpool.tile([128, GB, H], mybir.dt.float16, tag="zr")
                    zc = gpool.tile([128, GB, H], mybir.dt.float16, tag="zc")
                    nc.gpsimd.dma_gather(zr[:, :nb, :], zw_r[:, :],
                                         ir_all[:, o16:o16 + n16],
                                         n, n, H, transpose=False, single_packet=False,
                                         queue_num=next_q(n))
                    nc.gpsimd.dma_gather(zc[:, :nb, :], zw_c[:, :],
                                         ic_all[:, o16:o16 + n16],
                                         n, n, H, transpose=False, single_packet=False,
                                         queue_num=next_q(n))
                    if mode == "gather":
                        continue
                    for t in range(n // TILE):
                        S, pos = T // SUP, T % SUP
                        strip, k = pos // 32, pos % 32
                        if pos == 0:
                            out_ps = opp.tile([128, TILE], mybir.dt.float32, tag="ops")
                        # PE-transpose 4 blocks of each side into fp16 PSUM
                        trr = trp.tile([128, TILE], mybir.dt.float16, tag="trr")
                        trc = trp.tile([128, TILE], mybir.dt.float16, tag="trc")
                        for b in range(4):
                            nc.tensor.transpose(trr[:, b * 128:(b + 1) * 128],
                                                zr[:, 4 * t + b, :], ident_t[:])
                        for b in range(4):
                            nc.tensor.transpose(trc[:, b * 128:(b + 1) * 128],
                                                zc[:, 4 * t + b, :], ident_t[:])
                        zr_s = spool.tile([128, TILE], mybir.dt.float16, tag="zrs")
                        nc.vector.tensor_copy(zr_s[:], trr[:])
                        zc_s = spool.tile([128, TILE], mybir.dt.float16, tag="zcs")
                        nc.scalar.copy(zc_s[:], trc[:])
                        u_ps = upp.tile([128, TILE], mybir.dt.float32, tag="u")
                        nc.tensor.matmul(u_ps[:], w1a_t[:], zr_s[:], start=True, stop=False)
                        nc.tensor.matmul(u_ps[:], w1b_t[:], zc_s[:], start=False, stop=True)
                        y = ypool.tile([128, TILE], mybir.dt.float16, tag="y")
                        nc.scalar.activation(y[:], u_ps[:], mybir.ActivationFunctionType.Lrelu,
                                             bias=b1_t[:], scale=1.0, alpha=0.01)
                        last_in_strip = (k == 31) or (T == ntiles - 1)
                        nc.tensor.matmul(
                            out_ps[32 * strip:32 * (strip + 1), :],
                            w2e_t[:, 32 * k:32 * (k + 1)],
                            y[:],
                            start=(k == 0), stop=last_in_strip,
                            tile_position=(0, 32 * strip),
                        )
                        T += 1
                        if pos == SUP - 1 or T == ntiles:
                            rows = 32 * (strip + 1)
                            o_sb = opool.tile([128, TILE], mybir.dt.float32, tag="osb")
                            nc.vector.tensor_scalar_add(o_sb[:rows, :], out_ps[:rows, :], float(b2val))
                            nc.sync.dma_start(out=outd[S, 0:rows, :], in_=o_sb[:rows, :])

            if reps == 1:
                body()
            else:
                with tc.For_i(0, reps, 1) as _i:
                    body(_i)

    nc.compile()
    return nc


def prepare_inputs(z, edge_label_index, W1, b1, W2):
    z16 = np.asarray(z).astype(np.float16)
    eli = np.asarray(edge_label_index)
    row = eli[0].astype(np.int64)
    col = eli[1].astype(np.int64)
    caps, segments, tot, ntiles, nsup, per_core = _plan(row, col)

    W1 = np.asarray(W1, np.float32)
    w1aT16 = np.ascontiguousarray(W1[:, :H].T).astype(np.float16)
    w1bT16 = np.ascontiguousarray(W1[:, H:].T).astype(np.float16)
    b1_col = np.asarray(b1, np.float32).reshape(128, 1)
    w2_16 = np.asarray(W2, np.float32)[0].astype(np.float16)
    w2e_np = np.zeros((128, 32 * 32), np.float16)
    for k in range(32):
        w2e_np[:, 32 * k + k] = w2_16
    ident_np = np.eye(128, dtype=np.float16)

    group_start = np.zeros(NB * NB, np.int64)
    group_start[1:] = np.cumsum(caps)[:-1]

    in_maps = []
    scatter = []  # (order, valid_positions) per core
    for c in range(N_CORES):
        order, r_s, c_s, gs = per_core[c]
        # padded local indices, default 0 (gathers window base, discarded)
        lr = np.zeros(tot, np.int16)
        lc = np.zeros(tot, np.int16)
        sizes = np.bincount(gs, minlength=NB * NB)
        valid_pos = np.empty(EDGES_PER_CORE, np.int64)
        cur = 0
        for g in range(NB * NB):
            sgz = int(sizes[g])
            if sgz == 0:
                continue
            pos = group_start[g] + np.arange(sgz)
            lr[pos] = (r_s[cur:cur + sgz] - (g // NB) * BWIN).astype(np.int16)
            lc[pos] = (c_s[cur:cur + sgz] - (g % NB) * BWIN).astype(np.int16)
            valid_pos[cur:cur + sgz] = pos
            cur += sgz
        im_wins = {f"zw{w}": z16[w * BWIN:min((w + 1) * BWIN, N_NODES)] for w in range(NB)}
        in_maps.append({
            **im_wins,
            "idx_r": _wrap_idx(lr, tot),
            "idx_c": _wrap_idx(lc, tot),
            "w1aT": w1aT16, "w1bT": w1bT16,
            "b1d": b1_col, "w2e": w2e_np, "identd": ident_np,
        })
        scatter.append((order, valid_pos))
    return in_maps, scatter, segments, tot, ntiles, nsup


def assemble_output(results, scatter, nsup):
    out = np.empty(E_TOTAL, np.float32)
    for c in range(N_CORES):
        dev = results[c]["out"].reshape(nsup * 128 * TILE)
        order, valid_pos = scatter[c]
        oc = np.empty(EDGES_PER_CORE, np.float32)
        oc[order] = dev[valid_pos]
        out[c * EDGES_PER_CORE:(c + 1) * EDGES_PER_CORE] = oc
    return out


def kernel(z, edge_label_index, W1, b1, W2, b2):
    from concourse.bass_utils import run_bass_kernel_spmd

    in_maps, scatter, segments, tot, ntiles, nsup = prepare_inputs(
        z, edge_label_index, W1, b1, W2)
    b2val = float(np.asarray(b2).reshape(-1)[0])
    nc = build_program(ntiles, nsup, segments, b2val, reps=1)
    res = run_bass_kernel_spmd(nc, in_maps, list(range(N_CORES)))
    return assemble_output(res.results, scatter, nsup)
